# revision 3
# baseline (speedup 1.0000x reference)
"""Multi-head attention block (q/k/v projections + softmax attention +
out-projection) distributed over 8 TRN2 NeuronCores.

Sharding: core c handles batch b = c//2 and query rows [h*1024, (h+1)*1024),
h = c%2. Each core keeps the full kv of its batch (kv projections are
recomputed per query-half) so no inter-core collective is needed; the full
output is assembled host-side from disjoint shards.

Per-core layout (matmuls in fp32r; the kv chain in bf16 for SBUF headroom):
  qT/kvT   [model_dim, seq]  via PE transposes of natural-layout inputs
  qhT/khT  [inner, seq]      projection outputs, transposed layout
  vh       [seq_k, head*(64+1)] natural layout + ones column per head (the
                             ones column makes P@[V|1] produce the softmax
                             denominator for free)
  scores   S^T[k, q] on PSUM, two heads row-packed in the 128-row PE array
           (contraction dim HEAD_DIM=64 -> tile_position row tiling)
  exp      fused on ScalarE: exp(s/8), PSUM -> SBUF fp32r (no max-subtract:
           logits are O(5), exp is safe in fp32)
  PV       [65, q] PSUM accumulation over the 16 k tiles
  norm     denominator row -> Kc=1 ones-matmul broadcast -> fast reciprocal
           -> elementwise multiply
"""

import sys

sys.path.insert(0, "/opt/trn_rl_repo")

import numpy as np

B, NQ_FULL, NK = 4, 2048, 2048
NQ = 1024          # per-core query rows
DQ, DKV = 512, 768
HEADS, DH = 8, 64
INNER = 512
DA = DH + 1        # head dim + ones column
N_CORES = 8

_cache = {}


def _build():
    import concourse.bass as bass
    import concourse.tile as tile
    from concourse import bacc, mybir
    from concourse.masks import make_identity

    F32 = mybir.dt.float32
    F32R = mybir.dt.float32r
    BF16 = mybir.dt.bfloat16
    EXP = mybir.ActivationFunctionType.Exp

    nc = bacc.Bacc("TRN2", target_bir_lowering=False, debug=False,
                   enable_asserts=True, num_devices=N_CORES)

    q_d = nc.dram_tensor("q", [NQ, DQ], F32, kind="ExternalInput").ap()
    kv_d = nc.dram_tensor("kv", [NK, DKV], F32, kind="ExternalInput").ap()
    wq_d = nc.dram_tensor("Wq", [DQ, INNER], F32, kind="ExternalInput").ap()
    wk_d = nc.dram_tensor("Wk", [DKV, INNER], F32, kind="ExternalInput").ap()
    wv_d = nc.dram_tensor("Wv", [DKV, INNER], F32, kind="ExternalInput").ap()
    wo_d = nc.dram_tensor("Wo", [INNER, DQ], F32, kind="ExternalInput").ap()
    bo_d = nc.dram_tensor("bo", [DQ], F32, kind="ExternalInput").ap()
    out_d = nc.dram_tensor("out", [NQ, DQ], F32, kind="ExternalOutput").ap()

    MT_Q = DQ // 128      # 4  m-tiles of the q model dim
    MT_KV = DKV // 128    # 6  m-tiles of the kv model dim
    IT = INNER // 128     # 4  inner tiles (= head pairs)
    KT = NK // 128        # 16 key row tiles
    QB = NQ // 512        # 2  query blocks of 512
    PAIRS = HEADS // 2    # 4

    with tile.TileContext(nc) as tc:
        with (
            tc.tile_pool(name="consts", bufs=1) as consts,
            tc.tile_pool(name="wo", bufs=1) as wo_pool,
            tc.tile_pool(name="qhT", bufs=1) as qhT_pool,
            tc.tile_pool(name="khT", bufs=1) as khT_pool,
            tc.tile_pool(name="vh", bufs=1) as vh_pool,
            tc.tile_pool(name="attnT", bufs=1) as attnT_pool,
            tc.tile_pool(name="exps", bufs=3) as exps_pool,
            tc.tile_pool(name="outs", bufs=2) as outs_pool,
            tc.tile_pool(name="mm", bufs=2, space="PSUM") as ps_mm,
            tc.tile_pool(name="sc", bufs=2, space="PSUM") as ps_sc,
            tc.tile_pool(name="pv", bufs=2, space="PSUM") as ps_pv,
        ):
            # ---- constants ----
            ident = consts.tile([128, 128], F32)
            make_identity(nc, ident[:])
            ones1f = consts.tile([1, 64], F32)
            nc.vector.memset(ones1f[:], 1.0)
            ones1 = consts.tile([1, 64], F32R)
            nc.vector.tensor_copy(ones1[:], ones1f[:])
            ones8f = consts.tile([128, 8, 1], F32)
            nc.vector.memset(ones8f[:], 1.0)
            ones8 = consts.tile([128, 8, 1], F32R)
            nc.vector.tensor_copy(ones8[:], ones8f[:])
            bo_b = consts.tile([128, DQ], F32)
            nc.gpsimd.dma_start(
                out=bo_b[:],
                in_=bass.AP(tensor=bo_d.tensor, offset=bo_d.offset,
                            ap=[[0, 128]] + list(bo_d.ap)),
            )
            wo_r = wo_pool.tile([128, IT, 512], F32R, tag="wo")
            with tc.tile_pool(name="wst0", bufs=1) as wst0:
                st = wst0.tile([128, IT, 512], F32, tag="wst")
                nc.sync.dma_start(st[:], wo_d.rearrange("(t p) i -> p t i", p=128))
                nc.vector.tensor_copy(wo_r[:], st[:])

            qhT = [qhT_pool.tile([128, NQ], F32R, tag=f"qhT{it}", name=f"qhT{it}") for it in range(IT)]
            khT = [khT_pool.tile([128, NK], F32R, tag=f"khT{it}", name=f"khT{it}") for it in range(IT)]
            vh = [vh_pool.tile([128, HEADS, DA], F32R, tag=f"vh{kt}", name=f"vh{kt}") for kt in range(KT)]
            attnT = [attnT_pool.tile([128, NQ], F32R, tag=f"at{it}", name=f"at{it}") for it in range(IT)]

            # ================= q chain (fp32r) =================
            with (
                tc.tile_pool(name="wq", bufs=1) as wq_pool,
                tc.tile_pool(name="qT", bufs=1) as qT_pool,
            ):
                wq_r = wq_pool.tile([128, MT_Q, 512], F32R, tag="wq")
                with tc.tile_pool(name="wst1", bufs=1) as wst1:
                    st = wst1.tile([128, MT_Q, 512], F32, tag="wst")
                    nc.sync.dma_start(st[:], wq_d.rearrange("(t p) i -> p t i", p=128))
                    nc.vector.tensor_copy(wq_r[:], st[:])

                qT = [qT_pool.tile([128, NQ], F32R, tag=f"qT{mt}", name=f"qT{mt}") for mt in range(MT_Q)]
                with tc.tile_pool(name="qnat", bufs=1) as qnat_pool:
                    qnat = [qnat_pool.tile([128, 4, DQ], F32, tag=f"qn{g}", name=f"qn{g}")
                            for g in range(2)]
                    for g in range(2):
                        nc.sync.dma_start(
                            qnat[g][:],
                            q_d[g * 512:(g + 1) * 512, :]
                            .rearrange("(t p) i -> p t i", p=128))
                    for mt in range(MT_Q):
                        for g in range(2):
                            pt = ps_mm.tile([128, 512], F32, tag="mm")
                            for j in range(4):
                                nc.tensor.transpose(
                                    pt[:, j * 128:(j + 1) * 128],
                                    qnat[g][:, j, mt * 128:(mt + 1) * 128],
                                    ident[:])
                            nc.vector.tensor_copy(
                                qT[mt][:, g * 512:(g + 1) * 512], pt[:])

                # qhT[i, n] = sum_m Wq[m, i] qT[m, n]
                for it in range(IT):
                    for nb in range(QB):
                        pp = ps_mm.tile([128, 512], F32, tag="mm")
                        for mt in range(MT_Q):
                            nc.tensor.matmul(
                                pp[:],
                                wq_r[:, mt, it * 128:(it + 1) * 128],
                                qT[mt][:, nb * 512:(nb + 1) * 512],
                                start=(mt == 0), stop=(mt == MT_Q - 1))
                        nc.vector.tensor_copy(
                            qhT[it][:, nb * 512:(nb + 1) * 512], pp[:])

            # ================= kv chain (bf16) =================
            with (
                tc.tile_pool(name="wkv", bufs=1) as wkv_pool,
                tc.tile_pool(name="kvT", bufs=1) as kvT_pool,
            ):
                wk_b = wkv_pool.tile([128, MT_KV, 512], BF16, tag="wk")
                wv_b = wkv_pool.tile([128, MT_KV, 512], BF16, tag="wv")
                with tc.tile_pool(name="wst2", bufs=2) as wst2:
                    for wd, wt in ((wk_d, wk_b), (wv_d, wv_b)):
                        st = wst2.tile([128, MT_KV, 512], F32, tag="wst")
                        nc.sync.dma_start(st[:], wd.rearrange("(t p) i -> p t i", p=128))
                        nc.vector.tensor_copy(wt[:], st[:])

                kvT = [kvT_pool.tile([128, NK], BF16, tag=f"kvT{mt}", name=f"kvT{mt}")
                       for mt in range(MT_KV)]
                with tc.tile_pool(name="kvnat", bufs=2) as kvnat_pool:
                    for g in range(4):
                        kn = kvnat_pool.tile([128, 4, DKV], F32, tag="kn")
                        nc.sync.dma_start(
                            kn[:],
                            kv_d[g * 512:(g + 1) * 512, :]
                            .rearrange("(t p) i -> p t i", p=128))
                        for mt in range(MT_KV):
                            pt = ps_mm.tile([128, 512], F32, tag="mm")
                            for j in range(4):
                                nc.tensor.transpose(
                                    pt[:, j * 128:(j + 1) * 128],
                                    kn[:, j, mt * 128:(mt + 1) * 128],
                                    ident[:])
                            nc.vector.tensor_copy(
                                kvT[mt][:, g * 512:(g + 1) * 512], pt[:])

                # khT[i, k] = sum_m Wk[m, i] kvT[m, k]
                for it in range(IT):
                    for nb in range(NK // 512):
                        pp = ps_mm.tile([128, 512], F32, tag="mm")
                        for mt in range(MT_KV):
                            nc.tensor.matmul(
                                pp[:],
                                wk_b[:, mt, it * 128:(it + 1) * 128],
                                kvT[mt][:, nb * 512:(nb + 1) * 512],
                                start=(mt == 0), stop=(mt == MT_KV - 1))
                        nc.vector.tensor_copy(
                            khT[it][:, nb * 512:(nb + 1) * 512], pp[:])

                # vh[k, (h, d)] = sum_m kvT[m, k] Wv[m, (h, d)], plus ones col
                for kt in range(KT):
                    pp = ps_mm.tile([128, 512], F32, tag="mm")
                    for mt in range(MT_KV):
                        nc.tensor.matmul(
                            pp[:],
                            kvT[mt][:, kt * 128:(kt + 1) * 128],
                            wv_b[:, mt, :],
                            start=(mt == 0), stop=(mt == MT_KV - 1))
                    nc.vector.tensor_copy(
                        vh[kt][:, :, 0:DH],
                        pp[:].rearrange("p (h d) -> p h d", h=HEADS))
                    nc.vector.tensor_copy(vh[kt][:, :, DH:DA], ones8[:])

            # ================= attention =================
            for t in range(PAIRS):
                hA, hB = 2 * t, 2 * t + 1
                for qb in range(QB):
                    qs = slice(qb * 512, (qb + 1) * 512)
                    pvA = ps_pv.tile([DA, 512], F32, tag="pv")
                    pvB = ps_pv.tile([DA, 512], F32, tag="pv")
                    for kt in range(KT):
                        ks = slice(kt * 128, (kt + 1) * 128)
                        sc = ps_sc.tile([128, 1024], F32, tag="sc")
                        nc.tensor.matmul(
                            sc[:, 0:512],
                            khT[t][0:64, ks], qhT[t][0:64, qs],
                            start=True, stop=True, tile_position=(0, 0))
                        nc.tensor.matmul(
                            sc[:, 512:1024],
                            khT[t][64:128, ks], qhT[t][64:128, qs],
                            start=True, stop=True, tile_position=(64, 0))
                        ex = exps_pool.tile([128, 1024], F32R, tag="exp")
                        nc.scalar.activation(ex[:], sc[:], EXP,
                                             scale=float(DH) ** -0.5)
                        nc.tensor.matmul(pvA[:], vh[kt][:, hA, :], ex[:, 0:512],
                                         start=(kt == 0), stop=(kt == KT - 1))
                        nc.tensor.matmul(pvB[:], vh[kt][:, hB, :], ex[:, 512:1024],
                                         start=(kt == 0), stop=(kt == KT - 1))
                    # normalization: denom row -> bcast -> 1/x -> multiply
                    dsb = exps_pool.tile([1, 1024], F32R, tag="exp")
                    nc.vector.tensor_copy(dsb[0:1, 0:512], pvA[DH:DA, :])
                    nc.vector.tensor_copy(dsb[0:1, 512:1024], pvB[DH:DA, :])
                    db = ps_sc.tile([128, 1024], F32, tag="sc")
                    nc.tensor.matmul(db[0:64, 0:512], ones1[:], dsb[0:1, 0:512],
                                     start=True, stop=True)
                    nc.tensor.matmul(db[0:64, 512:1024], ones1[:],
                                     dsb[0:1, 512:1024], start=True, stop=True)
                    rb = exps_pool.tile([64, 1024], F32, tag="exp")
                    nc.vector.reciprocal_approx_fast(rb[:], db[0:64, :])
                    nc.vector.tensor_mul(attnT[t][0:64, qs],
                                         pvA[0:DH, :], rb[:, 0:512])
                    nc.vector.tensor_mul(attnT[t][64:128, qs],
                                         pvB[0:DH, :], rb[:, 512:1024])

            # ================= output projection + bias =================
            for nt in range(NQ // 128):
                ns = slice(nt * 128, (nt + 1) * 128)
                po = ps_mm.tile([128, 512], F32, tag="mm")
                for it in range(IT):
                    nc.tensor.matmul(po[:], attnT[it][:, ns], wo_r[:, it, :],
                                     start=(it == 0), stop=(it == IT - 1))
                ot = outs_pool.tile([128, DQ], F32, tag="ot")
                nc.vector.tensor_add(ot[:], po[:], bo_b[:])
                nc.sync.dma_start(out_d[ns, :], ot[:])

    nc.compile()
    return nc


def kernel(q, kv, Wq, Wk, Wv, Wo, bo):
    from concourse.bass_utils import run_bass_kernel_spmd

    q = np.asarray(q, dtype=np.float32)
    kv = np.asarray(kv, dtype=np.float32)
    Wq = np.ascontiguousarray(np.asarray(Wq, dtype=np.float32))
    Wk = np.ascontiguousarray(np.asarray(Wk, dtype=np.float32))
    Wv = np.ascontiguousarray(np.asarray(Wv, dtype=np.float32))
    Wo = np.ascontiguousarray(np.asarray(Wo, dtype=np.float32))
    bo = np.ascontiguousarray(np.asarray(bo, dtype=np.float32))

    if "nc" not in _cache:
        _cache["nc"] = _build()
    nc = _cache["nc"]

    in_maps = []
    for c in range(N_CORES):
        b, h = c // 2, c % 2
        in_maps.append({
            "q": np.ascontiguousarray(q[b, h * NQ:(h + 1) * NQ]),
            "kv": np.ascontiguousarray(kv[b]),
            "Wq": Wq, "Wk": Wk, "Wv": Wv, "Wo": Wo, "bo": bo,
        })
    res = run_bass_kernel_spmd(nc, in_maps, core_ids=list(range(N_CORES)))
    out = np.empty((B, NQ_FULL, DQ), dtype=np.float32)
    for c in range(N_CORES):
        b, h = c // 2, c % 2
        out[b, h * NQ:(h + 1) * NQ] = res.results[c]["out"]
    return out


# revision 5
# speedup vs baseline: 1.1728x; 1.1728x over previous
"""Multi-head attention block (q/k/v projections + softmax attention +
out-projection) distributed over 8 TRN2 NeuronCores.

Sharding: core c handles batch b = c//2 and query rows [h*1024, (h+1)*1024),
h = c%2. Each core keeps the full kv of its batch (kv projections are
recomputed per query-half) so no inter-core collective is needed; the full
output is assembled host-side from disjoint shards.

Per-core dataflow (activations bf16, normalization/out-proj fp32r):
  q/kv -> bf16 DRAM scratch (SWDGE cast-DMA, per-128-column slice)
       -> qT/kvT [model_dim, seq] in SBUF via HWDGE DMA-transpose
  projections (bf16 matmuls, fp32 PSUM):
       qhT/khT [inner, seq] transposed layout, vh [seq_k, head*(64+1)]
       natural layout with a ones column (P@[V|1] then yields the softmax
       denominator for free)
  attention per head-pair (two heads row-packed in the PE via tile_position,
  contraction dim HEAD_DIM=64):
       scores S^T[k, q] on PSUM -> exp(s/8) fused on ScalarE -> bf16
       -> PV [65, q] PSUM accumulation over the 16 k tiles
       -> denominator row -> Kc=1 ones-matmul broadcast -> fast reciprocal
       -> multiply -> attnT (fp32r)
  out-projection (fp32r) + broadcast bias.

k/q projections for later head-pairs are interleaved into the attention
k-loops as PE filler so the TensorEngine never idles long enough for the
HAM clock gate to re-throttle it to 1.2 GHz.
"""

import sys

sys.path.insert(0, "/opt/trn_rl_repo")

import numpy as np

B, NQ_FULL, NK = 4, 2048, 2048
NQ = 1024          # per-core query rows
DQ, DKV = 512, 768
HEADS, DH = 8, 64
INNER = 512
DA = DH + 1        # head dim + ones column
N_CORES = 8

_cache = {}


def _build():
    import concourse.bass as bass
    import concourse.tile as tile
    from concourse import bacc, mybir

    F32 = mybir.dt.float32
    F32R = mybir.dt.float32r
    BF16 = mybir.dt.bfloat16
    EXP = mybir.ActivationFunctionType.Exp

    nc = bacc.Bacc("TRN2", target_bir_lowering=False, debug=False,
                   enable_asserts=True, num_devices=N_CORES)

    q_d = nc.dram_tensor("q", [NQ, DQ], F32, kind="ExternalInput").ap()
    kv_d = nc.dram_tensor("kv", [NK, DKV], F32, kind="ExternalInput").ap()
    wq_d = nc.dram_tensor("Wq", [DQ, INNER], F32, kind="ExternalInput").ap()
    wk_d = nc.dram_tensor("Wk", [DKV, INNER], F32, kind="ExternalInput").ap()
    wv_d = nc.dram_tensor("Wv", [DKV, INNER], F32, kind="ExternalInput").ap()
    wo_d = nc.dram_tensor("Wo", [INNER, DQ], F32, kind="ExternalInput").ap()
    bo_d = nc.dram_tensor("bo", [DQ], F32, kind="ExternalInput").ap()
    out_d = nc.dram_tensor("out", [NQ, DQ], F32, kind="ExternalOutput").ap()

    MT_Q = DQ // 128      # 4
    MT_KV = DKV // 128    # 6
    IT = INNER // 128     # 4 inner tiles (= head pairs)
    KT = NK // 128        # 16
    QB = NQ // 512        # 2
    PAIRS = HEADS // 2    # 4

    with tile.TileContext(nc) as tc:
        with (
            tc.tile_pool(name="consts", bufs=1) as consts,
            tc.tile_pool(name="wpool", bufs=1) as wpool,
            tc.tile_pool(name="xT", bufs=1) as xT_pool,
            tc.tile_pool(name="proj", bufs=1) as proj_pool,
            tc.tile_pool(name="attnT", bufs=1) as attnT_pool,
            tc.tile_pool(name="exps", bufs=4) as exps_pool,
            tc.tile_pool(name="outs", bufs=2) as outs_pool,
            tc.tile_pool(name="dram", bufs=1, space="DRAM") as dram_pool,
            tc.tile_pool(name="mm", bufs=2, space="PSUM") as ps_mm,
            tc.tile_pool(name="sc", bufs=2, space="PSUM") as ps_sc,
            tc.tile_pool(name="pv", bufs=2, space="PSUM") as ps_pv,
        ):
            # ---- bf16 scratch in DRAM, cast per 128-column slice ----
            kv_bf = dram_pool.tile([NK, DKV], BF16, tag="kv_bf")
            q_bf = dram_pool.tile([NQ, DQ], BF16, tag="q_bf")
            for mt in range(MT_KV):
                cs = slice(mt * 128, (mt + 1) * 128)
                nc.gpsimd.dma_start(out=kv_bf[:, cs], in_=kv_d[:, cs])
            for mt in range(MT_Q):
                cs = slice(mt * 128, (mt + 1) * 128)
                nc.gpsimd.dma_start(out=q_bf[:, cs], in_=q_d[:, cs])

            # ---- transposed activations via DMA-transpose ----
            kvT = [xT_pool.tile([128, NK], BF16, tag=f"kvT{mt}", name=f"kvT{mt}")
                   for mt in range(MT_KV)]
            qT = [xT_pool.tile([128, NQ], BF16, tag=f"qT{mt}", name=f"qT{mt}")
                  for mt in range(MT_Q)]
            for mt in range(MT_KV):
                nc.sync.dma_start_transpose(
                    out=kvT[mt][:], in_=kv_bf[:, mt * 128:(mt + 1) * 128])
            for mt in range(MT_Q):
                nc.sync.dma_start_transpose(
                    out=qT[mt][:], in_=q_bf[:, mt * 128:(mt + 1) * 128])

            # ---- constants / weights ----
            ones1f = consts.tile([1, 64], F32)
            nc.vector.memset(ones1f[:], 1.0)
            ones1 = consts.tile([1, 64], F32R)
            nc.vector.tensor_copy(ones1[:], ones1f[:])
            ones8 = consts.tile([128, 8, 1], BF16)
            ones8f = consts.tile([128, 8, 1], F32)
            nc.vector.memset(ones8f[:], 1.0)
            nc.vector.tensor_copy(ones8[:], ones8f[:])
            bo_b = consts.tile([128, DQ], F32)
            nc.gpsimd.dma_start(
                out=bo_b[:],
                in_=bass.AP(tensor=bo_d.tensor, offset=bo_d.offset,
                            ap=[[0, 128]] + list(bo_d.ap)),
            )
            wk_b = wpool.tile([128, MT_KV, 512], BF16, tag="wk")
            wv_b = wpool.tile([128, MT_KV, 512], BF16, tag="wv")
            wq_b = wpool.tile([128, MT_Q, 512], BF16, tag="wq")
            wo_r = wpool.tile([128, IT, 512], F32R, tag="wo")
            with tc.tile_pool(name="wstage", bufs=2) as wstage:
                for wd, wt, mt in ((wk_d, wk_b, MT_KV), (wv_d, wv_b, MT_KV),
                                   (wq_d, wq_b, MT_Q), (wo_d, wo_r, IT)):
                    st = wstage.tile([128, mt, 512], F32, tag="wst")
                    nc.sync.dma_start(st[:], wd.rearrange("(t p) i -> p t i", p=128))
                    nc.vector.tensor_copy(wt[:], st[:])

            # ---- projection outputs ----
            qhT = [proj_pool.tile([128, NQ], BF16, tag=f"qhT{i}", name=f"qhT{i}")
                   for i in range(IT)]
            khT = [proj_pool.tile([128, NK], BF16, tag=f"khT{i}", name=f"khT{i}")
                   for i in range(IT)]
            vh = [proj_pool.tile([128, HEADS, DA], BF16, tag=f"vh{k}", name=f"vh{k}")
                  for k in range(KT)]
            attnT = [attnT_pool.tile([128, NQ], F32R, tag=f"at{i}", name=f"at{i}")
                     for i in range(IT)]

            def emit_khT(it, nb):
                pp = ps_mm.tile([128, 512], F32, tag="mm", name="pp")
                for mt in range(MT_KV):
                    nc.tensor.matmul(
                        pp[:], wk_b[:, mt, it * 128:(it + 1) * 128],
                        kvT[mt][:, nb * 512:(nb + 1) * 512],
                        start=(mt == 0), stop=(mt == MT_KV - 1))
                nc.vector.tensor_copy(khT[it][:, nb * 512:(nb + 1) * 512], pp[:])

            def emit_qhT(it, nb):
                pp = ps_mm.tile([128, 512], F32, tag="mm", name="pp")
                for mt in range(MT_Q):
                    nc.tensor.matmul(
                        pp[:], wq_b[:, mt, it * 128:(it + 1) * 128],
                        qT[mt][:, nb * 512:(nb + 1) * 512],
                        start=(mt == 0), stop=(mt == MT_Q - 1))
                nc.vector.tensor_copy(qhT[it][:, nb * 512:(nb + 1) * 512], pp[:])

            def emit_vh(kt):
                pp = ps_mm.tile([128, 512], F32, tag="mm", name="pp")
                for mt in range(MT_KV):
                    nc.tensor.matmul(
                        pp[:], kvT[mt][:, kt * 128:(kt + 1) * 128],
                        wv_b[:, mt, :],
                        start=(mt == 0), stop=(mt == MT_KV - 1))
                nc.vector.tensor_copy(
                    vh[kt][:, :, 0:DH],
                    pp[:].rearrange("p (h d) -> p h d", h=HEADS))
                nc.vector.tensor_copy(vh[kt][:, :, DH:DA], ones8[:])

            # pre-attention minimum: pair-0 projections and the first v tiles
            for nb in range(NK // 512):
                emit_khT(0, nb)
            for nb in range(QB):
                emit_qhT(0, nb)
            emit_vh(0)
            emit_vh(1)

            # PE filler work to interleave into each pair's attention loop
            fillers = {t: [] for t in range(PAIRS)}
            fillers[0] = ([(lambda kt=kt: emit_vh(kt)) for kt in range(2, KT)]
                          + [(lambda nb=nb: emit_khT(1, nb)) for nb in range(NK // 512)]
                          + [(lambda nb=nb: emit_qhT(1, nb)) for nb in range(QB)])
            for t in (1, 2):
                fillers[t] = ([(lambda nb=nb, it=t + 1: emit_khT(it, nb))
                               for nb in range(NK // 512)]
                              + [(lambda nb=nb, it=t + 1: emit_qhT(it, nb))
                                 for nb in range(QB)])

            # ---- attention ----
            for t in range(PAIRS):
                hA, hB = 2 * t, 2 * t + 1
                todo = fillers[t]
                fi = 0
                for qb in range(QB):
                    qs = slice(qb * 512, (qb + 1) * 512)
                    pvA = ps_pv.tile([DA, 512], F32, tag="pv", name="pvA")
                    pvB = ps_pv.tile([DA, 512], F32, tag="pv", name="pvB")
                    for kt in range(KT):
                        ks = slice(kt * 128, (kt + 1) * 128)
                        sc = ps_sc.tile([128, 1024], F32, tag="sc", name="sc")
                        nc.tensor.matmul(
                            sc[:, 0:512],
                            khT[t][0:64, ks], qhT[t][0:64, qs],
                            start=True, stop=True, tile_position=(0, 0))
                        nc.tensor.matmul(
                            sc[:, 512:1024],
                            khT[t][64:128, ks], qhT[t][64:128, qs],
                            start=True, stop=True, tile_position=(64, 0))
                        ex = exps_pool.tile([128, 1024], BF16, tag="exp", name="ex")
                        nc.scalar.activation(ex[:], sc[:], EXP,
                                             scale=float(DH) ** -0.5)
                        nc.tensor.matmul(pvA[:], vh[kt][:, hA, :], ex[:, 0:512],
                                         start=(kt == 0), stop=(kt == KT - 1))
                        nc.tensor.matmul(pvB[:], vh[kt][:, hB, :], ex[:, 512:1024],
                                         start=(kt == 0), stop=(kt == KT - 1))
                        if fi < len(todo):
                            todo[fi]()
                            fi += 1
                    # normalization: denom row -> bcast -> 1/x -> multiply
                    dsb = exps_pool.tile([1, 1024], F32R, tag="exp", name="dsb")
                    nc.vector.tensor_copy(dsb[0:1, 0:512], pvA[DH:DA, :])
                    nc.vector.tensor_copy(dsb[0:1, 512:1024], pvB[DH:DA, :])
                    db = ps_sc.tile([128, 1024], F32, tag="sc", name="db")
                    nc.tensor.matmul(db[0:64, 0:512], ones1[:], dsb[0:1, 0:512],
                                     start=True, stop=True)
                    nc.tensor.matmul(db[0:64, 512:1024], ones1[:],
                                     dsb[0:1, 512:1024], start=True, stop=True)
                    rb = exps_pool.tile([64, 1024], F32, tag="exp", name="rb")
                    nc.vector.reciprocal_approx_fast(rb[:], db[0:64, :])
                    nc.vector.tensor_mul(attnT[t][0:64, qs],
                                         pvA[0:DH, :], rb[:, 0:512])
                    nc.vector.tensor_mul(attnT[t][64:128, qs],
                                         pvB[0:DH, :], rb[:, 512:1024])
                while fi < len(todo):
                    todo[fi]()
                    fi += 1

            # ---- output projection + bias ----
            for nt in range(NQ // 128):
                ns = slice(nt * 128, (nt + 1) * 128)
                po = ps_mm.tile([128, 512], F32, tag="mm", name="po")
                for it in range(IT):
                    nc.tensor.matmul(po[:], attnT[it][:, ns], wo_r[:, it, :],
                                     start=(it == 0), stop=(it == IT - 1))
                ot = outs_pool.tile([128, DQ], F32, tag="ot", name="ot")
                nc.vector.tensor_add(ot[:], po[:], bo_b[:])
                nc.sync.dma_start(out_d[ns, :], ot[:])

    nc.compile()
    return nc


def kernel(q, kv, Wq, Wk, Wv, Wo, bo):
    from concourse.bass_utils import run_bass_kernel_spmd

    q = np.asarray(q, dtype=np.float32)
    kv = np.asarray(kv, dtype=np.float32)
    Wq = np.ascontiguousarray(np.asarray(Wq, dtype=np.float32))
    Wk = np.ascontiguousarray(np.asarray(Wk, dtype=np.float32))
    Wv = np.ascontiguousarray(np.asarray(Wv, dtype=np.float32))
    Wo = np.ascontiguousarray(np.asarray(Wo, dtype=np.float32))
    bo = np.ascontiguousarray(np.asarray(bo, dtype=np.float32))

    if "nc" not in _cache:
        _cache["nc"] = _build()
    nc = _cache["nc"]

    in_maps = []
    for c in range(N_CORES):
        b, h = c // 2, c % 2
        in_maps.append({
            "q": np.ascontiguousarray(q[b, h * NQ:(h + 1) * NQ]),
            "kv": np.ascontiguousarray(kv[b]),
            "Wq": Wq, "Wk": Wk, "Wv": Wv, "Wo": Wo, "bo": bo,
        })
    res = run_bass_kernel_spmd(nc, in_maps, core_ids=list(range(N_CORES)))
    out = np.empty((B, NQ_FULL, DQ), dtype=np.float32)
    for c in range(N_CORES):
        b, h = c // 2, c % 2
        out[b, h * NQ:(h + 1) * NQ] = res.results[c]["out"]
    return out
